# revision 1
# baseline (speedup 1.0000x reference)
"""MoE expert-parallel FFN kernel for Trainium2 (8 NeuronCores).

Problem: x [4, 16384, 1024]; 8 experts, expert e applies
    y = gelu(x_chunk @ w1[e] + b1[e]) @ w2[e] + b2[e]
to tokens [e*2048:(e+1)*2048] of every group (chunk along dim 1).

Sharding: expert-parallel, one expert per core. Each core runs an
identical program on its own x chunk (8192 tokens) and expert weights.
No collectives.

Per-core kernel layout (all matmuls in f32r = full-rate fp32):
  host passes xT = x_chunk.T  [D, T] so both matmuls need no on-device
  transposes:
    mm1: hT[f,t]  = w1[d,f].T @ xT[d,t]   (lhsT = w1 tile, rhs = xT tile)
    mm2: yT[d,t]  = w2[f,d].T @ hT[f,t]   (lhsT = w2 tile, rhs = hT tile)
  d_ff (4096) is split in two resident phases of 2048 (w1+w2 halves =
  16 MB SBUF); each phase streams all tokens; phase 0 writes partial
  yT to a DRAM scratch, phase 1 adds its contribution and writes yT.
"""

import os
import sys

import numpy as np

for _p in ("/opt/trn_rl_repo", "/root/.axon_site/_ro/trn_rl_repo"):
    if os.path.isdir(_p) and _p not in sys.path:
        sys.path.insert(0, _p)

import concourse.bass as bass  # noqa: E402
import concourse.tile as tile  # noqa: E402
from concourse import bacc, mybir  # noqa: E402
from concourse.bass_utils import run_bass_kernel_spmd  # noqa: E402

# Problem shape (hardcoded per contract)
E = 8          # experts == cores
G = 4          # groups
TFULL = 16384  # tokens per group
D = 1024       # d_model
F = 4096       # d_ff
C = TFULL // E     # tokens per expert chunk per group (2048)
T = G * C          # tokens per core (8192)

TB = 512           # token block (matmul free dim)
NTB = T // TB      # 16
FBLK = 2048        # d_ff per phase
NPH = F // FBLK    # 2
KD = D // 128      # 8  k-tiles over d_model
MF = FBLK // 128   # 16 d_ff tiles per phase
MD = D // 128      # 8  d_model output tiles

f32 = mybir.dt.float32
f32r = mybir.dt.float32r

_NC_CACHE = {}


def _build_nc():
    nc = bacc.Bacc()
    xT = nc.dram_tensor("xT", [D, T], f32r, kind="ExternalInput")
    w1 = nc.dram_tensor("w1", [D, F], f32r, kind="ExternalInput")
    b1 = nc.dram_tensor("b1", [F], f32, kind="ExternalInput")
    w2 = nc.dram_tensor("w2", [F, D], f32r, kind="ExternalInput")
    b2 = nc.dram_tensor("b2", [D], f32, kind="ExternalInput")
    yT = nc.dram_tensor("yT", [D, T], f32, kind="ExternalOutput")

    xTr = xT.rearrange("(k p) t -> p k t", p=128)    # [128, KD, T]
    w1r = w1.rearrange("(k p) f -> p k f", p=128)    # [128, KD, F]
    w2r = w2.rearrange("(m p) d -> p m d", p=128)    # [128, F//128, D]
    b1r = b1.rearrange("(m p) -> p m", p=128)        # [128, F//128]
    b2r = b2.rearrange("(m p) -> p m", p=128)        # [128, MD]

    gelu = mybir.ActivationFunctionType.Gelu

    with tile.TileContext(nc) as tc:
        with tc.tile_pool(name="wpool", bufs=1) as wpool, \
             tc.tile_pool(name="xpool", bufs=2) as xpool, \
             tc.tile_pool(name="hpool", bufs=1) as hpool, \
             tc.tile_pool(name="ypool", bufs=3) as ypool, \
             tc.tile_pool(name="bpool", bufs=1) as bpool, \
             tc.tile_pool(name="dram", bufs=1, space="DRAM") as dpool, \
             tc.tile_pool(name="psum", bufs=2, space="PSUM") as psum:

            y0 = dpool.tile([D, T], f32)
            b2t = bpool.tile([128, MD], f32)
            nc.sync.dma_start(b2t, b2r)

            for ph in range(NPH):
                w1t = wpool.tile([128, KD, FBLK], f32r, tag="w1t")
                nc.sync.dma_start(w1t, w1r[:, :, ph * FBLK:(ph + 1) * FBLK])
                w2t = wpool.tile([128, MF, D], f32r, tag="w2t")
                nc.sync.dma_start(w2t, w2r[:, ph * MF:(ph + 1) * MF, :])
                b1t = bpool.tile([128, MF], f32, tag="b1t")
                nc.sync.dma_start(b1t, b1r[:, ph * MF:(ph + 1) * MF])

                for tb in range(NTB):
                    t0 = tb * TB
                    xt = xpool.tile([128, KD, TB], f32r, tag="xt")
                    nc.sync.dma_start(xt, xTr[:, :, t0:t0 + TB])

                    ht = hpool.tile([128, MF, TB], f32r, tag="ht")
                    for m in range(MF):
                        ps = psum.tile([128, TB], f32, tag="ps1")
                        for k in range(KD):
                            nc.tensor.matmul(
                                ps,
                                lhsT=w1t[:, k, m * 128:(m + 1) * 128],
                                rhs=xt[:, k, :],
                                start=(k == 0),
                                stop=(k == KD - 1),
                            )
                        nc.scalar.activation(ht[:, m, :], ps, gelu,
                                             bias=b1t[:, m:m + 1])

                    for mo in range(MD):
                        ps2 = psum.tile([128, TB], f32, tag="ps2")
                        for m in range(MF):
                            nc.tensor.matmul(
                                ps2,
                                lhsT=w2t[:, m, mo * 128:(mo + 1) * 128],
                                rhs=ht[:, m, :],
                                start=(m == 0),
                                stop=(m == MF - 1),
                            )
                        if ph == 0:
                            yt = ypool.tile([128, TB], f32, tag="yt")
                            nc.vector.tensor_scalar_add(yt, ps2,
                                                        b2t[:, mo:mo + 1])
                            nc.sync.dma_start(
                                y0[mo * 128:(mo + 1) * 128, t0:t0 + TB], yt)
                        else:
                            y0t = ypool.tile([128, TB], f32, tag="y0t")
                            nc.sync.dma_start(
                                y0t, y0[mo * 128:(mo + 1) * 128, t0:t0 + TB])
                            yt = ypool.tile([128, TB], f32, tag="yt")
                            nc.vector.tensor_add(yt, ps2, y0t)
                            nc.sync.dma_start(
                                yT[mo * 128:(mo + 1) * 128, t0:t0 + TB], yt)

    nc.compile()
    return nc


def _get_nc():
    if "nc" not in _NC_CACHE:
        _NC_CACHE["nc"] = _build_nc()
    return _NC_CACHE["nc"]


def kernel(x, w1, b1, w2, b2, _trace=False, _trace_kwargs=None):
    x = np.asarray(x, dtype=np.float32)
    w1 = np.asarray(w1, dtype=np.float32)
    b1 = np.asarray(b1, dtype=np.float32)
    w2 = np.asarray(w2, dtype=np.float32)
    b2 = np.asarray(b2, dtype=np.float32)

    nc = _get_nc()
    xe = x.reshape(G, E, C, D)
    in_maps = []
    for e in range(E):
        xc = np.ascontiguousarray(xe[:, e].reshape(T, D).T)  # [D, T]
        in_maps.append({
            "xT": xc,
            "w1": np.ascontiguousarray(w1[e]),
            "b1": np.ascontiguousarray(b1[e]),
            "w2": np.ascontiguousarray(w2[e]),
            "b2": np.ascontiguousarray(b2[e]),
        })

    kw = dict(_trace_kwargs or {})
    res = run_bass_kernel_spmd(nc, in_maps, list(range(E)), trace=_trace, **kw)

    out = np.empty((G, TFULL, D), dtype=np.float32)
    for e in range(E):
        yTv = res.results[e]["yT"]                    # [D, T]
        out[:, e * C:(e + 1) * C, :] = yTv.T.reshape(G, C, D)

    if _trace:
        kernel.last_exec_time_ns = res.exec_time_ns
        kernel.last_results = res
    return out


# revision 8
# speedup vs baseline: 3.7994x; 3.7994x over previous
"""MoE expert-parallel FFN kernel for Trainium2 (8 NeuronCores).

Problem: x [4, 16384, 1024]; 8 experts, expert e applies
    y = gelu(x_chunk @ w1[e] + b1[e]) @ w2[e] + b2[e]
to tokens [e*2048:(e+1)*2048] of every group (chunk along dim 1).

Sharding: expert-parallel, one expert per core. Each core runs an
identical program on its own x chunk (8192 tokens) and expert weights.
No collectives.

Per-core kernel layout (all matmuls in f32r = full-rate fp32):
  host passes xT = x_chunk.T  [D, T] so both matmuls need no on-device
  transposes:
    mm1: hT[f,t]  = w1[d,f].T @ xT[d,t]   (lhsT = w1 tile, rhs = xT tile)
    mm2: yT[d,t]  = w2[f,d].T @ hT[f,t]   (lhsT = w2 tile, rhs = hT tile)
  d_ff (4096) is split in two resident phases of 2048 (w1+w2 halves =
  16 MB SBUF); each phase streams all tokens; phase 0 writes partial
  yT to a DRAM scratch, phase 1 adds its contribution and writes yT.
"""

import os
import sys

import numpy as np

for _p in ("/opt/trn_rl_repo", "/root/.axon_site/_ro/trn_rl_repo"):
    if os.path.isdir(_p) and _p not in sys.path:
        sys.path.insert(0, _p)

import concourse.bass as bass  # noqa: E402
import concourse.tile as tile  # noqa: E402
from concourse import bacc, mybir  # noqa: E402
from concourse.bass_utils import run_bass_kernel_spmd  # noqa: E402

# Problem shape (hardcoded per contract)
E = 8          # experts == cores
G = 4          # groups
TFULL = 16384  # tokens per group
D = 1024       # d_model
F = 4096       # d_ff
C = TFULL // E     # tokens per expert chunk per group (2048)
T = G * C          # tokens per core (8192)

TB = 512           # token block (matmul free dim)
NTB = T // TB      # 16
FBLK = 2048        # d_ff per phase
NPH = F // FBLK    # 2
KD = D // 128      # 8  k-tiles over d_model
MF = FBLK // 128   # 16 d_ff tiles per phase
MD = D // 128      # 8  d_model output tiles

f32 = mybir.dt.float32
f32r = mybir.dt.float32r

_NC_CACHE = {}


def _build_nc(repeats=1, psum_bufs=4, y_bufs=3, x_first=True):
    nc = bacc.Bacc()
    xT = nc.dram_tensor("xT", [D, T], f32r, kind="ExternalInput")
    w1 = nc.dram_tensor("w1", [D, F], f32r, kind="ExternalInput")
    b1 = nc.dram_tensor("b1", [F], f32, kind="ExternalInput")
    w2 = nc.dram_tensor("w2", [F, D], f32r, kind="ExternalInput")
    b2 = nc.dram_tensor("b2", [D], f32, kind="ExternalInput")
    yT = nc.dram_tensor("yT", [D, T], f32, kind="ExternalOutput")

    xTr = xT.rearrange("(k p) t -> p k t", p=128)    # [128, KD, T]
    w1r = w1.rearrange("(k p) f -> p k f", p=128)    # [128, KD, F]
    w2r = w2.rearrange("(m p) d -> p m d", p=128)    # [128, F//128, D]
    b1r = b1.rearrange("(m p) -> p m", p=128)        # [128, F//128]
    b2r = b2.rearrange("(m p) -> p m", p=128)        # [128, MD]

    gelu = mybir.ActivationFunctionType.Gelu

    with tile.TileContext(nc) as tc:
        with tc.tile_pool(name="wpool", bufs=1) as wpool, \
             tc.tile_pool(name="xpool", bufs=2) as xpool, \
             tc.tile_pool(name="hpool", bufs=1) as hpool, \
             tc.tile_pool(name="ypool", bufs=y_bufs) as ypool, \
             tc.tile_pool(name="bpool", bufs=1) as bpool, \
             tc.tile_pool(name="dram", bufs=1, space="DRAM") as dpool, \
             tc.tile_pool(name="psum", bufs=psum_bufs, space="PSUM") as psum:

            y0 = dpool.tile([D, T], f32)
            b2t = bpool.tile([128, MD], f32)
            nc.sync.dma_start(b2t, b2r)

            for ph in [p for _ in range(repeats) for p in range(NPH)]:
                w1t = wpool.tile([128, KD, FBLK], f32r, tag="w1t")
                nc.sync.dma_start(w1t, w1r[:, :, ph * FBLK:(ph + 1) * FBLK])
                b1t = bpool.tile([128, MF], f32, tag="b1t")
                nc.sync.dma_start(b1t, b1r[:, ph * MF:(ph + 1) * MF])
                xt0 = None
                if x_first:
                    # first token block's x before w2: mm1 can start sooner
                    xt0 = xpool.tile([128, KD, TB], f32r, tag="xt")
                    nc.sync.dma_start(xt0, xTr[:, :, 0:TB])
                w2t = wpool.tile([128, MF, D], f32r, tag="w2t")
                nc.sync.dma_start(w2t, w2r[:, ph * MF:(ph + 1) * MF, :])

                for tb in range(NTB):
                    t0 = tb * TB
                    if tb == 0 and xt0 is not None:
                        xt = xt0
                    else:
                        xt = xpool.tile([128, KD, TB], f32r, tag="xt")
                        nc.sync.dma_start(xt, xTr[:, :, t0:t0 + TB])

                    ht = hpool.tile([128, MF, TB], f32r, tag="ht")
                    for m in range(MF):
                        ps = psum.tile([128, TB], f32, tag="ps1")
                        for k in range(KD):
                            nc.tensor.matmul(
                                ps,
                                lhsT=w1t[:, k, m * 128:(m + 1) * 128],
                                rhs=xt[:, k, :],
                                start=(k == 0),
                                stop=(k == KD - 1),
                            )
                        nc.scalar.activation(ht[:, m, :], ps, gelu,
                                             bias=b1t[:, m:m + 1])

                    for mo in range(MD):
                        ps2 = psum.tile([128, TB], f32, tag="ps2")
                        for m in range(MF):
                            nc.tensor.matmul(
                                ps2,
                                lhsT=w2t[:, m, mo * 128:(mo + 1) * 128],
                                rhs=ht[:, m, :],
                                start=(m == 0),
                                stop=(m == MF - 1),
                            )
                        if ph == 0:
                            yt = ypool.tile([128, TB], f32, tag="yt")
                            nc.vector.tensor_scalar_add(yt, ps2,
                                                        b2t[:, mo:mo + 1])
                            nc.sync.dma_start(
                                y0[mo * 128:(mo + 1) * 128, t0:t0 + TB], yt)
                        else:
                            y0t = ypool.tile([128, TB], f32, tag="y0t")
                            nc.sync.dma_start(
                                y0t, y0[mo * 128:(mo + 1) * 128, t0:t0 + TB])
                            yt = ypool.tile([128, TB], f32, tag="yt")
                            nc.vector.tensor_add(yt, ps2, y0t)
                            nc.sync.dma_start(
                                yT[mo * 128:(mo + 1) * 128, t0:t0 + TB], yt)

    nc.compile()
    return nc


def _get_nc():
    if "nc" not in _NC_CACHE:
        _NC_CACHE["nc"] = _build_nc()
    return _NC_CACHE["nc"]


def kernel(x, w1, b1, w2, b2, _trace=False, _trace_kwargs=None):
    x = np.asarray(x, dtype=np.float32)
    w1 = np.asarray(w1, dtype=np.float32)
    b1 = np.asarray(b1, dtype=np.float32)
    w2 = np.asarray(w2, dtype=np.float32)
    b2 = np.asarray(b2, dtype=np.float32)

    nc = _get_nc()
    xe = x.reshape(G, E, C, D)
    in_maps = []
    for e in range(E):
        xc = np.ascontiguousarray(xe[:, e].reshape(T, D).T)  # [D, T]
        in_maps.append({
            "xT": xc,
            "w1": np.ascontiguousarray(w1[e]),
            "b1": np.ascontiguousarray(b1[e]),
            "w2": np.ascontiguousarray(w2[e]),
            "b2": np.ascontiguousarray(b2[e]),
        })

    kw = dict(_trace_kwargs or {})
    try:
        res = run_bass_kernel_spmd(nc, in_maps, list(range(E)),
                                   trace=_trace, **kw)
    except Exception:
        # transient device wedge (e.g. NRT_EXEC_UNIT_UNRECOVERABLE) — retry
        res = run_bass_kernel_spmd(nc, in_maps, list(range(E)),
                                   trace=_trace, **kw)

    out = np.empty((G, TFULL, D), dtype=np.float32)
    for e in range(E):
        yTv = res.results[e]["yT"]                    # [D, T]
        out[:, e * C:(e + 1) * C, :] = yTv.T.reshape(G, C, D)

    if _trace:
        kernel.last_exec_time_ns = res.exec_time_ns
        kernel.last_results = res
    return out
